# revision 13
# baseline (speedup 1.0000x reference)
"""
Mixture-of-Experts (top-2 of 8, SwiGLU experts) Trainium2 Bass kernel.

Strategy: expert-parallel across 8 NeuronCores. Core e owns expert e's
weights (Wg/Wu/Wd transposed host-side to contraction-major layout) and
computes, over ALL T=4096 tokens:
  - the full gating (logits -> softmax -> top-2 -> normalized gates) from a
    replicated xT + per-core permuted gate weights (the permutation rotates
    the core's own expert into column 0, so the SPMD program is identical
    across cores),
  - the load-balance loss (identical on every core; host takes core 0's),
  - out_partial[h, t] = w_e[t] * ((x@WgT) * silu(x@WuT) @ WdT)[h, t]
    where w_e[t] is this expert's normalized gate (exactly 0 for tokens not
    routed here).
Host sums the 8 partials (exact: non-routed partials are exactly zero) and
transposes back to [B, S, H].

All matmuls run as float32r (full-rate fp32 on the PE at free-dim>=256);
the tiny gating matmuls run as plain float32 for maximum precision since
discrete top-2 decisions depend on logit order.
"""

import numpy as np

import concourse.bass as bass
import concourse.mybir as mybir
import concourse.tile as tile
from concourse import bacc
from concourse.bass_utils import run_bass_kernel_spmd
from concourse.masks import make_identity

F32 = mybir.dt.float32
F32R = mybir.dt.float32r
AX = mybir.AxisListType
ALU = mybir.AluOpType
ACTF = mybir.ActivationFunctionType

# Problem dims (hardcoded; harness runs this exact shape).
B, S, H, E, F, TOP_K = 2, 2048, 1024, 8, 4096, 2
T = B * S

# Tiling (chosen to fit 192KB/partition SBUF and 8 PSUM banks).
T_CHUNK = 1024   # tokens per outer chunk
T_SUB = 512      # psum tile free dim (one fp32 bank)
F_HALF = 2048    # f processed per yT residency
F_GROUP = 128    # f per Wg/Wu weight-slab DMA

N_CORES = 8

# Set by test.py to capture a profile.
TRACE = False
LAST_RESULTS = {}


def build_moe(Hd=H, Fd=F, Td=T, Ed=E, t_chunk=T_CHUNK, t_sub=T_SUB,
              f_half=F_HALF, f_group=F_GROUP, mm_dtype=F32R):
    nh = Hd // 128
    n_chunk = Td // t_chunk
    n_ts = t_chunk // t_sub
    n_tt = t_chunk // 128
    n_fh = Fd // f_half
    nf_h = f_half // 128
    n_fg = f_half // f_group
    nf_g = f_group // 128

    nc = bacc.Bacc("TRN2", target_bir_lowering=False, debug=False)

    xT = nc.dram_tensor("xT", [Hd, Td], mm_dtype, kind="ExternalInput").ap()
    if mm_dtype == F32R:
        xTg = xT.bitcast(F32)  # f32r bits are f32 bits; gating reads raw x
    else:
        xTg = nc.dram_tensor("xTg", [Hd, Td], F32, kind="ExternalInput").ap()
    gwT = nc.dram_tensor("gwT", [Hd, Ed], F32, kind="ExternalInput").ap()
    wgT = nc.dram_tensor("wgT", [Hd, Fd], mm_dtype, kind="ExternalInput").ap()
    wuT = nc.dram_tensor("wuT", [Hd, Fd], mm_dtype, kind="ExternalInput").ap()
    wdT = nc.dram_tensor("wdT", [Fd, Hd], mm_dtype, kind="ExternalInput").ap()
    out = nc.dram_tensor("out", [Hd, Td], F32, kind="ExternalOutput").ap()
    loss = nc.dram_tensor("loss", [1, 1], F32, kind="ExternalOutput").ap()

    def r128(ap, pat):
        return ap.rearrange(pat, p=128)

    with tile.TileContext(nc) as tc:
        with (
            tc.tile_pool(name="xt", bufs=1) as p_xt,
            tc.tile_pool(name="wgu", bufs=2) as p_wgu,
            tc.tile_pool(name="wd", bufs=2) as p_wd,
            tc.tile_pool(name="yt", bufs=1) as p_yt,
            tc.tile_pool(name="accs", bufs=1) as p_acc,
            tc.tile_pool(name="sil", bufs=2) as p_sil,
            tc.tile_pool(name="gate", bufs=2) as p_gate,
            tc.tile_pool(name="persist", bufs=1) as p_per,
            tc.tile_pool(name="pgu", bufs=2, space="PSUM") as pp_gu,
            tc.tile_pool(name="pacc", bufs=2, space="PSUM") as pp_acc,
            tc.tile_pool(name="pgate", bufs=1, space="PSUM") as pp_gate,
        ):
            # --- persistent small tiles ---
            ident = p_per.tile([128, 128], F32, tag="ident")
            make_identity(nc, ident)
            ones1 = p_per.tile([1, 128], F32, tag="ones1")
            nc.vector.memset(ones1, 1.0)
            ones128 = p_per.tile([128, 1], F32, tag="ones128")
            nc.vector.memset(ones128, 1.0)
            gw_sb = p_per.tile([128, nh, Ed], F32, tag="gw")
            nc.sync.dma_start(out=gw_sb, in_=r128(gwT, "(a p) e -> p a e"))
            macc = p_per.tile([128, Ed], F32, tag="macc")
            nc.vector.memset(macc, 0.0)

            for ci in range(n_chunk):
                tsl = slice(ci * t_chunk, (ci + 1) * t_chunk)
                xt = p_xt.tile([128, nh, t_chunk], mm_dtype, tag="xt")
                nc.sync.dma_start(out=xt, in_=r128(xT[:, tsl], "(a p) t -> p a t"))

                # ---------- gating (separate f32 x tiles: top-2 selection
                # must match the fp32 reference ordering exactly) ----------
                w_row = p_gate.tile([1, t_chunk], F32, tag="wrow")
                for tt in range(n_tt):
                    xg = p_gate.tile([128, nh, 128], F32, tag="xg")
                    t0 = ci * t_chunk + tt * 128
                    nc.sync.dma_start(
                        out=xg,
                        in_=r128(xTg[:, t0:t0 + 128], "(a p) t -> p a t"))
                    plog = pp_gate.tile([128, Ed], F32, tag="plog")
                    for a in range(nh):
                        nc.tensor.matmul(
                            plog,
                            lhsT=xg[:, a, :],
                            rhs=gw_sb[:, a, :],
                            start=(a == 0), stop=(a == nh - 1),
                        )
                    nmx = p_gate.tile([128, 1], F32, tag="nmx")
                    nc.vector.tensor_reduce(nmx, plog, axis=AX.X, op=ALU.max,
                                            negate=True)
                    esb = p_gate.tile([128, Ed], F32, tag="esb")
                    ssum = p_gate.tile([128, 1], F32, tag="ssum")
                    nc.scalar.activation(esb, plog, ACTF.Exp, bias=nmx,
                                         scale=1.0, accum_out=ssum)
                    rs = p_gate.tile([128, 1], F32, tag="rs")
                    nc.vector.reciprocal(rs, ssum)
                    pt = p_gate.tile([128, Ed], F32, tag="pt")
                    nc.vector.tensor_scalar_mul(pt, esb, rs)
                    m1 = p_gate.tile([128, 1], F32, tag="m1")
                    nc.vector.tensor_reduce(m1, pt, axis=AX.X, op=ALU.max)
                    mask1 = p_gate.tile([128, Ed], F32, tag="mask1")
                    nc.vector.tensor_scalar(mask1, pt, m1, None, op0=ALU.is_ge)
                    mnot1 = p_gate.tile([128, Ed], F32, tag="mnot1")
                    nc.vector.tensor_scalar(mnot1, pt, m1, None, op0=ALU.is_lt)
                    pm = p_gate.tile([128, Ed], F32, tag="pm")
                    nc.vector.tensor_mul(pm, pt, mnot1)
                    m2 = p_gate.tile([128, 1], F32, tag="m2")
                    nc.vector.tensor_reduce(m2, pm, axis=AX.X, op=ALU.max)
                    mask2 = p_gate.tile([128, Ed], F32, tag="mask2")
                    nc.vector.tensor_scalar(mask2, pm, m2, None, op0=ALU.is_ge)
                    m12 = p_gate.tile([128, Ed], F32, tag="m12")
                    nc.vector.tensor_add(m12, mask1, mask2)
                    nc.vector.tensor_add(macc, macc, m12)
                    den = p_gate.tile([128, 1], F32, tag="den")
                    nc.vector.tensor_scalar(den, m1, m2, 1e-8,
                                            op0=ALU.add, op1=ALU.add)
                    rden = p_gate.tile([128, 1], F32, tag="rden")
                    nc.vector.reciprocal(rden, den)
                    g1 = p_gate.tile([128, 1], F32, tag="g1")
                    nc.vector.tensor_mul(g1, m1, rden)
                    g2 = p_gate.tile([128, 1], F32, tag="g2")
                    nc.vector.tensor_mul(g2, m2, rden)
                    wa = p_gate.tile([128, 1], F32, tag="wa")
                    nc.vector.tensor_mul(wa, mask1[:, 0:1], g1)
                    wb = p_gate.tile([128, 1], F32, tag="wb")
                    nc.vector.tensor_mul(wb, mask2[:, 0:1], g2)
                    wcol = p_gate.tile([128, 1], F32, tag="wcol")
                    nc.vector.tensor_add(wcol, wa, wb)
                    pwrow = pp_gate.tile([1, 128], F32, tag="plog")
                    nc.tensor.matmul(pwrow, lhsT=wcol, rhs=ident,
                                     start=True, stop=True)
                    nc.scalar.copy(w_row[0:1, tt * 128:(tt + 1) * 128], pwrow)

                # broadcast w_row across partitions via K=1 matmul
                w_bc = p_gate.tile([128, t_chunk], F32, tag="wbc")
                for si in range(n_ts):
                    ssl = slice(si * t_sub, (si + 1) * t_sub)
                    pbc = pp_gate.tile([128, t_sub], F32, tag="pbc")
                    nc.tensor.matmul(pbc, lhsT=ones1, rhs=w_row[0:1, ssl],
                                     start=True, stop=True)
                    nc.vector.tensor_copy(w_bc[:, ssl], pbc)

                # ---------- expert FFN ----------
                acc_sb = p_acc.tile([128, nh, t_chunk], F32, tag="acc_sb")
                for fh in range(n_fh):
                    yt = p_yt.tile([128, nf_h, t_chunk], mm_dtype, tag="yt")
                    # phase A: yT[f, t] = (Wg x) * silu(Wu x)
                    for fg in range(n_fg):
                        f0 = fh * f_half + fg * f_group
                        fsl = slice(f0, f0 + f_group)
                        wg_sb = p_wgu.tile([128, nh, f_group], mm_dtype, tag="wg")
                        nc.sync.dma_start(out=wg_sb,
                                          in_=r128(wgT[:, fsl], "(a p) f -> p a f"))
                        wu_sb = p_wgu.tile([128, nh, f_group], mm_dtype, tag="wu")
                        nc.sync.dma_start(out=wu_sb,
                                          in_=r128(wuT[:, fsl], "(a p) f -> p a f"))
                        for k in range(nf_g):
                            ft = fg * nf_g + k
                            ksl = slice(k * 128, (k + 1) * 128)
                            for si in range(n_ts):
                                ssl = slice(si * t_sub, (si + 1) * t_sub)
                                pg = pp_gu.tile([128, t_sub], F32, tag="pg")
                                for a in range(nh):
                                    nc.tensor.matmul(
                                        pg,
                                        lhsT=wg_sb[:, a, ksl],
                                        rhs=xt[:, a, ssl],
                                        start=(a == 0), stop=(a == nh - 1),
                                    )
                                pu = pp_gu.tile([128, t_sub], F32, tag="pu")
                                for a in range(nh):
                                    nc.tensor.matmul(
                                        pu,
                                        lhsT=wu_sb[:, a, ksl],
                                        rhs=xt[:, a, ssl],
                                        start=(a == 0), stop=(a == nh - 1),
                                    )
                                sil = p_sil.tile([128, t_sub], F32, tag="sil")
                                nc.scalar.activation(sil, pu, ACTF.Sigmoid)
                                us = p_sil.tile([128, t_sub], F32, tag="us")
                                nc.vector.tensor_mul(us, pu, sil)
                                nc.vector.tensor_mul(yt[:, ft, ssl], pg, us)
                    # phase B: acc[h, t] += WdT.T @ yT  (contract f over this half)
                    for a in range(nh):
                        hsl = slice(a * 128, (a + 1) * 128)
                        wd_sb = p_wd.tile([128, nf_h, 128], mm_dtype, tag="wd")
                        nc.sync.dma_start(
                            out=wd_sb,
                            in_=r128(wdT[fh * f_half:(fh + 1) * f_half, hsl],
                                     "(a p) h -> p a h"))
                        for si in range(n_ts):
                            ssl = slice(si * t_sub, (si + 1) * t_sub)
                            pacc = pp_acc.tile([128, t_sub], F32, tag="acc")
                            for k in range(nf_h):
                                nc.tensor.matmul(
                                    pacc,
                                    lhsT=wd_sb[:, k, :],
                                    rhs=yt[:, k, ssl],
                                    start=(k == 0), stop=(k == nf_h - 1),
                                )
                            if fh == 0:
                                nc.vector.tensor_copy(acc_sb[:, a, ssl], pacc)
                            else:
                                nc.vector.tensor_add(acc_sb[:, a, ssl],
                                                     acc_sb[:, a, ssl], pacc)

                # weight by this expert's gate and store
                for a in range(nh):
                    for si in range(n_ts):
                        ssl = slice(si * t_sub, (si + 1) * t_sub)
                        nc.vector.tensor_mul(acc_sb[:, a, ssl],
                                             acc_sb[:, a, ssl], w_bc[:, ssl])
                nc.sync.dma_start(out=r128(out[:, tsl], "(a p) t -> p a t"),
                                  in_=acc_sb)

            # ---------- load-balance loss ----------
            pcnt = pp_gate.tile([1, Ed], F32, tag="plog")
            nc.tensor.matmul(pcnt, lhsT=ones128, rhs=macc, start=True, stop=True)
            u = p_gate.tile([1, Ed], F32, tag="u")
            nc.scalar.mul(u, pcnt, 1.0 / (Td * TOP_K))
            d = p_gate.tile([1, Ed], F32, tag="d")
            nc.vector.tensor_scalar_add(d, u, -1.0 / Ed)
            d2 = p_gate.tile([1, Ed], F32, tag="d2")
            nc.vector.tensor_mul(d2, d, d)
            sv = p_gate.tile([1, 1], F32, tag="sv")
            nc.vector.tensor_reduce(sv, d2, axis=AX.X, op=ALU.add)
            r = p_gate.tile([1, 1], F32, tag="r")
            nc.scalar.mul(r, sv, (1.0 / (Ed - 1)) / (1.0 / Ed + 1e-8))
            l2 = p_gate.tile([1, 1], F32, tag="l2")
            nc.vector.tensor_mul(l2, r, r)
            nc.sync.dma_start(out=loss, in_=l2)

    nc.compile()
    return nc


_PROG_CACHE = {}

# fp16 (e5m10) mantissa is close to fp32r's e8m11 but the PE weight path is
# FWL-eligible: measured 1.72ms vs 1.97ms for fp32r at 4.6e-4 vs 2.3e-4 rel
# err (gating stays fp32 either way, so routing/loss are exact). Default fp16.
import os  # noqa: E402
USE_FP16 = os.environ.get("MOE_FP16", "1") == "1"
MM_DT = mybir.dt.float16 if USE_FP16 else F32R
MM_NP = np.float16 if USE_FP16 else np.float32


def _get_program():
    if "nc" not in _PROG_CACHE:
        _PROG_CACHE["nc"] = build_moe(mm_dtype=MM_DT)
    return _PROG_CACHE["nc"]


def make_in_maps(x, gate_w, Wg, Wu, Wd, mm_np=np.float32):
    x = np.asarray(x, np.float32)
    gate_w = np.asarray(gate_w, np.float32)
    Wg = np.asarray(Wg, np.float32)
    Wu = np.asarray(Wu, np.float32)
    Wd = np.asarray(Wd, np.float32)
    xf = np.ascontiguousarray(x.reshape(T, H).T)  # [H, T]
    xf_mm = xf if mm_np == np.float32 else xf.astype(mm_np)
    in_maps = []
    for e in range(N_CORES):
        perm = np.roll(np.arange(E), -e)
        m = {
            "xT": xf_mm,
            "gwT": np.ascontiguousarray(gate_w[perm].T),
            "wgT": np.ascontiguousarray(Wg[e].T).astype(mm_np, copy=False),
            "wuT": np.ascontiguousarray(Wu[e].T).astype(mm_np, copy=False),
            "wdT": np.ascontiguousarray(Wd[e].T).astype(mm_np, copy=False),
        }
        if mm_np != np.float32:
            m["xTg"] = xf
        in_maps.append(m)
    return in_maps


def kernel(x, gate_w, Wg, Wu, Wd):
    nc = _get_program()
    in_maps = make_in_maps(x, gate_w, Wg, Wu, Wd, mm_np=MM_NP)
    trace = TRACE
    if trace:
        try:
            from antenv.axon_hooks import get_axon_ntff_profile_hook  # noqa: F401
        except ImportError:
            trace = False
    res = run_bass_kernel_spmd(nc, in_maps, list(range(N_CORES)), trace=trace)
    LAST_RESULTS["res"] = res
    total = np.zeros((H, T), np.float64)
    for e in range(N_CORES):
        total += res.results[e]["out"]
    out = np.ascontiguousarray(total.T.astype(np.float32)).reshape(B, S, H)
    loss = np.float32(res.results[0]["loss"].reshape(())[()])
    return out, loss
